# revision 1
# baseline (speedup 1.0000x reference)
"""Trainium2 Bass kernel for CausalSelfAttention2D.

Math (per batch element b):
  xn = ChannelLayerNorm(x)          # over C per spatial position
  qkv = qkv_w @ xn + qkv_b          # 1x1 conv == matmul over C
  per head h: S = (q_h^T k_h)/8 ; causal mask ; P = softmax(S)
  O_h = v_h @ P^T ; out = proj_w @ concat(O) + proj_b

Sharding: data-parallel over B (8 batch elements -> 8 cores), identical
SPMD program per core.

Host-side algebraic folds (exact):
  - ln_g folded into qkv_w columns; ln_b folded into qkv_b.
  - v-part of qkv bias folded into proj_b (softmax rows sum to 1).
  - pos_h/pos_w additive per-head scalar bias is a softmax no-op (masked
    entries are set to -FLT_MAX *after* the bias in the reference), so it
    is dropped.

On-chip layout (per core):
  x, xn:  [C=512, L=1024] as 4 tiles of [128, 1024]   (C on partitions)
  LN stats via ones-matmul column sums (partition reduction on PE).
  q, k:   [512, L] 4 tiles [128, 1152] (128 cols zero pad for i-padding)
  vT:     [L, 512] 8 tiles [128, 512] in bf16
  scores: computed transposed, S^T[j, i], per head pair (row-packed
          K=64 matmuls via tile_position); exp on ACT (scale=1/8) straight
          out of PSUM into bf16 P^T tiles; causal mask applied post-exp as
          a 0/1 triangular multiply on the diagonal 128-col block.
  AV + den: col-packed (tile_position) matmuls over j-tiles; denominator
          via ones-lhsT matmul producing a [64,i] broadcast; out = AV *
          recip(den).
  proj:   [512, 512] @ O.

Matmul dtype: float32r (TF32-like full-rate fp32 path) except AV/den which
use bf16 P^T / vT.
"""

import os
import sys
import numpy as np

import concourse.bass as bass
import concourse.mybir as mybir
import concourse.tile as tile
from concourse import bacc
from concourse.bass import ds, ts
from concourse.bass_utils import run_bass_kernel_spmd


F32 = mybir.dt.float32
F32R = mybir.dt.float32r
BF16 = mybir.dt.bfloat16
FP16 = mybir.dt.float16

B, C, H, W = 8, 512, 32, 32
L = H * W                      # 1024
HEADS = 8
DM = 512
DH = 64                        # d_head
EPS = 1e-5
NCORES = 8

LPAD = L

# scores^T chunking per j-tile t: list of (i_start, n_cols);每 chunk stays
# inside one 512-col PSUM bank of the per-head mega region.
ST_CHUNKS = {
    0: [(0, 512), (512, 512)],
    1: [(128, 512), (640, 384)],
    2: [(256, 512), (768, 256)],
    3: [(384, 512), (896, 128)],
    4: [(512, 512)],
    5: [(640, 384)],
    6: [(768, 256)],
    7: [(896, 128)],
}
ST_EXT = {t: chunks[-1][0] + chunks[-1][1] - 128 * t for t, chunks in ST_CHUNKS.items()}


def _emit(nc, tc):
    x_d = nc.dram_tensor("x", [C, L], FP16, kind="ExternalInput").ap()
    wqkvT_d = nc.dram_tensor("wqkvT", [C, 3 * DM], FP16, kind="ExternalInput").ap()
    bq_d = nc.dram_tensor("bq", [DM], F32, kind="ExternalInput").ap()
    bk_d = nc.dram_tensor("bk", [DM], F32, kind="ExternalInput").ap()
    wprojT_d = nc.dram_tensor("wprojT", [DM, C], FP16, kind="ExternalInput").ap()
    onescol_d = nc.dram_tensor("onescol", [128, 1], FP16, kind="ExternalInput").ap()
    onesrow_d = nc.dram_tensor("onesrow", [1, 128], FP16, kind="ExternalInput").ap()
    bproj_d = nc.dram_tensor("bproj", [C], F32, kind="ExternalInput").ap()
    y_d = nc.dram_tensor("y", [C, L], F32, kind="ExternalOutput").ap()

    fexp = mybir.ActivationFunctionType.Exp
    fsqrt = mybir.ActivationFunctionType.Sqrt
    fcopy = mybir.ActivationFunctionType.Copy

    with (
        tc.tile_pool(name="const", bufs=1) as cpool,
        tc.tile_pool(name="pers", bufs=1) as pers,
        tc.tile_pool(name="pT", bufs=10) as ppool,
    ):
        # ---- constants ----
        ones_col = cpool.tile([128, 1], FP16, tag="ones_col")
        nc.sync.dma_start(ones_col[:], onescol_d[:])
        ones_row = cpool.tile([1, 128], FP16, tag="ones_row")
        nc.sync.dma_start(ones_row[:], onesrow_d[:])
        ones_den = cpool.tile([128, DH], FP16, tag="ones_den")
        nc.gpsimd.memset(ones_den[:], 1.0)
        # tri[p, f] = 1.0 if f >= p else 0.0   (keep i_rel >= j_rel)
        eps128 = cpool.tile([128, 1], F32, tag="eps")
        nc.gpsimd.memset(eps128[:], EPS)
        tri = cpool.tile([128, 128], FP16, tag="tri")
        nc.gpsimd.memset(tri[:], 1.0)
        nc.gpsimd.affine_select(
            out=tri[:], in_=tri[:],
            compare_op=mybir.AluOpType.is_ge,
            fill=0.0, base=0, pattern=[[1, 128]], channel_multiplier=-1,
        )

        # ---- persistent tiles ----
        q_t = [pers.tile([128, LPAD], FP16, tag=f"q{m}", name=f"q{m}") for m in range(4)]
        k_t = [pers.tile([128, L], FP16, tag=f"k{m}", name=f"k{m}") for m in range(4)]
        # vT_ext[j, 128h:128h+64] = v^T head h; cols 128h+64:128h+128 = 1.0
        # so one [128,128] stationary computes AV (rows 0-63) and the
        # softmax denominator broadcast (rows 64-127) in a single matmul.
        vT_t = [pers.tile([128, 2 * DM], FP16, tag=f"vT{m}", name=f"vT{m}") for m in range(8)]
        o_t = [pers.tile([128, L], FP16, tag=f"o{m}", name=f"o{m}") for m in range(4)]
        wproj_t = [pers.tile([128, C], FP16, tag=f"wp{m}", name=f"wp{m}") for m in range(4)]
        bq_t = [pers.tile([128, 1], F32, tag=f"bq{m}", name=f"bq{m}") for m in range(4)]
        bk_t = [pers.tile([128, 1], F32, tag=f"bk{m}", name=f"bk{m}") for m in range(4)]
        bp_t = [pers.tile([128, 1], F32, tag=f"bp{m}", name=f"bp{m}") for m in range(4)]

        for m in range(4):
            nc.sync.dma_start(bq_t[m][:], bq_d[ds(m * 128, 128)].rearrange("(p o) -> p o", o=1))
            nc.sync.dma_start(bk_t[m][:], bk_d[ds(m * 128, 128)].rearrange("(p o) -> p o", o=1))
            nc.sync.dma_start(bp_t[m][:], bproj_d[ds(m * 128, 128)].rearrange("(p o) -> p o", o=1))
            nc.sync.dma_start(wproj_t[m][:], wprojT_d[ts(m, 128), :])

        # =========== Phase A: LayerNorm ===========
        with (
            tc.tile_pool(name="xa", bufs=1) as xpool,
            tc.tile_pool(name="sqa", bufs=2) as sqpool,
            tc.tile_pool(name="rows", bufs=1) as rpool,
            tc.tile_pool(name="xn", bufs=1) as xnpool,
            tc.tile_pool(name="wq", bufs=1) as wqpool,
            tc.tile_pool(name="psW", bufs=1, space="PSUM") as psW,
            tc.tile_pool(name="psA", bufs=3, space="PSUM") as psA,
            tc.tile_pool(name="psAb", bufs=2, space="PSUM") as psAb,
            tc.tile_pool(name="psB", bufs=2, space="PSUM") as psB,
        ):
            x_t = [xpool.tile([128, L], FP16, tag=f"x{c}", name=f"x{c}") for c in range(4)]
            for ch in range(2):
                eng = nc.sync if ch == 0 else nc.gpsimd
                for c in range(4):
                    eng.dma_start(
                        x_t[c][:, ts(ch, 512)], x_d[ts(c, 128), ts(ch, 512)]
                    )
            w_t = [wqpool.tile([128, 3 * DM], FP16, tag=f"w{c}", name=f"w{c}") for c in range(4)]
            for c in range(4):
                eng = nc.sync if c % 2 == 0 else nc.gpsimd
                eng.dma_start(w_t[c][:], wqkvT_d[ts(c, 128), :])

            # PE warmup: bursts of dep-free matmuls keep the HAM clock-gate
            # open (2.4 GHz) across the LN/DMA head where PE would idle.
            def warmup(n):
                wu = psW.tile([64, 64], F32, tag="wu", name="wu")
                for _ in range(n):
                    nc.tensor.matmul(wu[:], ones_den[:], ones_den[:],
                                     start=True, stop=True)

            warmup(64)

            # column sums of x and x^2 -> [1, 1024] stats
            sum_ps = [psA.tile([1, 512], F32, tag="stat", name=f"sum_ps{i}") for i in range(2)]
            sq_ps = [psA.tile([1, 512], F32, tag="stat", name=f"sq_ps{i}") for i in range(2)]
            sq_t = []
            for c in range(4):
                sq = sqpool.tile([128, L], FP16, tag="sq")
                for ch in range(2):
                    nc.vector.tensor_mul(
                        sq[:, ts(ch, 512)], x_t[c][:, ts(ch, 512)], x_t[c][:, ts(ch, 512)]
                    )
                sq_t.append(sq)
            for ch in range(2):
                for c in range(4):
                    nc.tensor.matmul(
                        sum_ps[ch][:], (ones_col[:]), (x_t[c][:, ts(ch, 512)]),
                        start=(c == 0), stop=(c == 3),
                    )
                for c in range(4):
                    nc.tensor.matmul(
                        sq_ps[ch][:], (ones_col[:]), (sq_t[c][:, ts(ch, 512)]),
                        start=(c == 0), stop=(c == 3),
                    )

            warmup(24)

            # stats chain in [128, 8] layout (1-partition ops are ~100x
            # slower per element; bounce through an SBUF->SBUF DMA reshape)
            stats_row = rpool.tile([1, 2 * L], F32, tag="statrow")
            s_row = rpool.tile([1, L], FP16, tag="s")
            t_row = rpool.tile([1, L], FP16, tag="t")
            for ch in range(2):
                nc.scalar.activation(stats_row[:, ts(ch, 512)], sum_ps[ch][:], fcopy, scale=1.0 / C)
                nc.scalar.activation(stats_row[:, ds(L + 512 * ch, 512)], sq_ps[ch][:], fcopy, scale=1.0 / C)
            st = rpool.tile([128, 16], F32, tag="st")       # cols 0-7 mu, 8-15 msq
            nc.sync.dma_start(st[:, ds(0, 8)], stats_row[ds(0, 1), ds(0, L)])
            nc.sync.dma_start(st[:, ds(8, 8)], stats_row[ds(0, 1), ds(L, L)])
            mu2 = rpool.tile([128, 8], F32, tag="mu2")
            nc.vector.tensor_mul(mu2[:], st[:, ds(0, 8)], st[:, ds(0, 8)])
            nc.vector.tensor_sub(mu2[:], st[:, ds(8, 8)], mu2[:])   # var
            nc.scalar.activation(mu2[:], mu2[:], fsqrt, bias=eps128[:])
            srec = rpool.tile([128, 8], F32, tag="srec")
            nc.vector.reciprocal_approx_fast(srec[:], mu2[:])
            s16 = rpool.tile([128, 16], FP16, tag="s16")    # cols 0-7 s, 8-15 t
            nc.vector.tensor_copy(s16[:, ds(0, 8)], srec[:])
            nc.vector.tensor_mul(s16[:, ds(8, 8)], st[:, ds(0, 8)], srec[:])
            nc.sync.dma_start(s_row[ds(0, 1), :], s16[:, ds(0, 8)])
            nc.sync.dma_start(t_row[ds(0, 1), :], s16[:, ds(8, 8)])

            # broadcast s,t down 128 partitions via K=1 matmul
            bs_t = rpool.tile([128, L], FP16, tag="bs")
            bt_t = rpool.tile([128, L], FP16, tag="bt")
            for ch in range(2):
                for row, dst in ((s_row, bs_t), (t_row, bt_t)):
                    ps = psAb.tile([128, 512], F32, tag="bc")
                    nc.tensor.matmul(ps[:], (ones_row[:]), (row[:, ts(ch, 512)]),
                                     start=True, stop=True)
                    nc.vector.tensor_copy(dst[:, ts(ch, 512)], ps[:])

            warmup(24)

            xn_t = []
            for c in range(4):
                xn = xnpool.tile([128, L], FP16, tag=f"xn{c}")
                nc.vector.tensor_mul(xn[:], x_t[c][:], bs_t[:])
                nc.vector.tensor_sub(xn[:], xn[:], bt_t[:])
                xn_t.append(xn)

            # =========== Phase B: qkv projections ===========
            # q[m], k[m]: [128, 1024]; vT[m8]: [128(l), 512] bf16
            for m in range(4):
                for ch in range(2):
                    for name, off, dst, bias in (
                        ("q", 0, q_t[m], bq_t[m]),
                        ("k", DM, k_t[m], bk_t[m]),
                    ):
                        ps = psB.tile([128, 512], F32, tag="mm")
                        for c in range(4):
                            nc.tensor.matmul(
                                ps[:],
                                (w_t[c][:, ds(off + m * 128, 128)]),
                                (xn_t[c][:, ts(ch, 512)]),
                                start=(c == 0), stop=(c == 3),
                            )
                        nc.vector.tensor_scalar_add(dst[:, ts(ch, 512)], ps[:], bias[:])
            for m8 in range(8):
                for h in range(8):
                    nc.gpsimd.memset(vT_t[m8][:, ds(128 * h + 64, 64)], 1.0)
                ps = psB.tile([128, 512], F32, tag="mm")
                for c in range(4):
                    nc.tensor.matmul(
                        ps[:],
                        (xn_t[c][:, ts(m8, 128)]),
                        (w_t[c][:, ds(2 * DM, DM)]),
                        start=(c == 0), stop=(c == 3),
                    )
                for h in range(8):
                    nc.vector.tensor_copy(
                        vT_t[m8][:, ds(128 * h, 64)], ps[:, ds(64 * h, 64)]
                    )

        # =========== Phase C: attention per head pair ===========
        with (
            tc.tile_pool(name="psT", bufs=3, space="PSUM") as psT,
            tc.tile_pool(name="psAV", bufs=2, space="PSUM") as psAV,
            tc.tile_pool(name="rsb", bufs=2) as rsb,
        ):
            for p in range(4):  # head pair (2p, 2p+1)
                pT_tiles = {}
                for t in range(8):
                    ext = ST_EXT[t]
                    i0 = 128 * t
                    pT = ppool.tile([128, 2048], FP16, tag="pT")
                    megas = []
                    for hh in range(2):  # head within pair
                        megas.append(psT.tile([128, 1024], F32, tag="sT",
                                              name=f"sT{p}_{t}_{hh}"))
                    # chunk-major, head-minor: consecutive matmuls hit
                    # disjoint PE row-groups and overlap in the array
                    for (ist, ncols) in ST_CHUNKS[t]:
                        for hh in range(2):
                            pb = 64 * hh
                            nc.tensor.matmul(
                                megas[hh][:, ds(ist - i0, ncols)],
                                (k_t[p][ds(pb, 64), ts(t, 128)]),
                                (q_t[p][ds(pb, 64), ds(ist, ncols)]),
                                start=True, stop=True,
                                tile_position=(pb, 0),
                            )
                    for hh in range(2):
                        nc.scalar.activation(
                            pT[:, ds(hh * 1024, ext)],
                            megas[hh][:, ds(0, ext)],
                            fexp, scale=0.125,
                        )
                        # causal mask on the diagonal 128-col block
                        # (GpSimd: idle during attention; DVE is loaded)
                        nc.gpsimd.tensor_mul(
                            pT[:, ds(hh * 1024, 128)], pT[:, ds(hh * 1024, 128)], tri[:]
                        )
                    pT_tiles[t] = pT

                # AV + denominator in one matmul per (head, chunk, j-tile):
                # stationary [vT_h | ones] -> rows 0-63 AV, rows 64-127 den
                for cch in range(2):
                    tlist = range(4) if cch == 0 else range(8)
                    avs = []
                    for hh in range(2):
                        h = 2 * p + hh
                        av = psAV.tile([128, 512], F32, tag="av",
                                       name=f"av{p}_{cch}_{hh}")
                        avs.append(av)
                        for ti, t in enumerate(tlist):
                            lo = max(cch * 512, 128 * t)
                            n = (cch + 1) * 512 - lo
                            nc.tensor.matmul(
                                av[:, ds(lo - cch * 512, n)],
                                vT_t[t][:, ds(128 * h, 128)],
                                pT_tiles[t][:, ds(hh * 1024 + lo - 128 * t, n)],
                                start=(ti == 0), stop=(ti == len(tlist) - 1),
                            )
                    for hh in range(2):
                        rec = rsb.tile([128, 512], F32, tag="rec")
                        nc.vector.reciprocal_approx_fast(rec[:], avs[hh][:, :])
                        nc.vector.tensor_mul(
                            o_t[p][ds(64 * hh, 64), ts(cch, 512)],
                            avs[hh][ds(0, 64), :], rec[ds(64, 64), :],
                        )

            # =========== Phase D: output projection ===========
            for m in range(4):
                yt = rsb.tile([128, L], F32, tag="y")
                for ch in range(2):
                    ps = psAV.tile([128, 512], F32, tag="av")
                    for c2 in range(4):
                        nc.tensor.matmul(
                            ps[:],
                            (wproj_t[c2][:, ts(m, 128)]),
                            (o_t[c2][:, ts(ch, 512)]),
                            start=(c2 == 0), stop=(c2 == 3),
                        )
                    nc.vector.tensor_scalar_add(yt[:, ts(ch, 512)], ps[:], bp_t[m][:])
                nc.sync.dma_start(y_d[ts(m, 128), :], yt[:])


_NC_CACHE = None


def build_nc():
    global _NC_CACHE
    if _NC_CACHE is None:
        nc = bacc.Bacc("TRN2", target_bir_lowering=False, debug=False)
        with tile.TileContext(nc) as tc:
            _emit(nc, tc)
        nc.compile()
        _NC_CACHE = nc
    return _NC_CACHE


def host_inputs(x, ln_g, ln_b, qkv_w, qkv_b, proj_w, proj_b, pos_h, pos_w):
    """Fold LN affine + v-bias; build per-core input maps."""
    x = np.asarray(x, np.float32)
    ln_g = np.asarray(ln_g, np.float32)
    ln_b = np.asarray(ln_b, np.float32)
    qkv_w = np.asarray(qkv_w, np.float32)
    qkv_b = np.asarray(qkv_b, np.float32)
    proj_w = np.asarray(proj_w, np.float32)
    proj_b = np.asarray(proj_b, np.float32)

    w_eff = qkv_w * ln_g[None, :]                    # [1536, 512]
    b_eff = qkv_b + qkv_w @ ln_b                     # [1536]
    wqkvT = np.ascontiguousarray(w_eff.T)            # [512, 1536]
    bq, bk, bv = b_eff[:DM], b_eff[DM:2 * DM], b_eff[2 * DM:]
    bproj = proj_b + proj_w @ bv                     # [512]
    wprojT = np.ascontiguousarray(proj_w.T)          # [512, 512]

    common = {
        "wqkvT": wqkvT.astype(np.float16), "bq": np.ascontiguousarray(bq),
        "bk": np.ascontiguousarray(bk),
        "wprojT": wprojT.astype(np.float16),
        "bproj": np.ascontiguousarray(bproj),
        "onescol": np.ones((128, 1), np.float16),
        "onesrow": np.ones((1, 128), np.float16),
    }
    in_maps = []
    for b in range(B):
        m = dict(common)
        m["x"] = np.ascontiguousarray(x[b].reshape(C, L)).astype(np.float16)
        in_maps.append(m)
    return in_maps


def kernel(x, ln_g, ln_b, qkv_w, qkv_b, proj_w, proj_b, pos_h, pos_w, **kw):
    nc = build_nc()
    in_maps = host_inputs(x, ln_g, ln_b, qkv_w, qkv_b, proj_w, proj_b, pos_h, pos_w)
    res = run_bass_kernel_spmd(nc, in_maps, core_ids=list(range(NCORES)))
    out = np.stack([res.results[b]["y"].reshape(C, H, W) for b in range(B)])
    return out.astype(np.float32)


if __name__ == "__main__":
    nc = build_nc()
    print("built + compiled ok")



# revision 18
# speedup vs baseline: 1.3141x; 1.3141x over previous
"""Trainium2 Bass kernel for CausalSelfAttention2D.

Math (per batch element b):
  xn = ChannelLayerNorm(x)          # over C per spatial position
  qkv = qkv_w @ xn + qkv_b          # 1x1 conv == matmul over C
  per head h: S = (q_h^T k_h)/8 ; causal mask ; P = softmax(S)
  O_h = v_h @ P^T ; out = proj_w @ concat(O) + proj_b

Sharding: data-parallel over B (8 batch elements -> 8 cores), identical
SPMD program per core.

Host-side algebraic folds (exact):
  - ln_g folded into qkv_w columns; ln_b folded into qkv_b.
  - v-part of qkv bias folded into proj_b (softmax rows sum to 1).
  - k bias dropped entirely: terms it adds to scores depend only on the
    query index -> cancel in softmax.
  - pos_h/pos_w additive per-head scalar bias is a softmax no-op.

LayerNorm is applied AFTER the qkv matmul so the PE never waits on the
stats chain:
  q = (sum_c W x  + negmu (x) wsumq + sigma (x) bq) * s[l]
  (negmu/sigma rows enter the PSUM accumulation as K=2 rank-2 matmul;
   per-position scale s multiplies on DVE straight out of PSUM.)
Stats come from ones-matmul column sums of x and x^2 (PE), a short
[1,1024] row chain (ACT/DVE), two tiny SBUF DMAs to stack (negmu, sigma)
into a [2,1024] tile for the K=2 correction matmuls.

Attention: scores computed transposed per head pair (row-packed K=64
matmuls), exp on ACT over both heads at once (3-dim AP into chunk-pair
PSUM tiles), causal mask as a 0/1 triangular multiply on DVE (4x mode),
AV + softmax denominator fused via [v | ones] stationary blocks.
"""

import numpy as np

import concourse.bass as bass
import concourse.mybir as mybir
import concourse.tile as tile
from concourse import bacc
from concourse.bass import ds, ts
from concourse.bass_utils import run_bass_kernel_spmd


F32 = mybir.dt.float32
FP16 = mybir.dt.float16

B, C, H, W = 8, 512, 32, 32
L = H * W                      # 1024
HEADS = 8
DM = 512
DH = 64                        # d_head
NCORES = 8

# scores^T chunking per j-tile t: list of (i_start, n_cols); each chunk
# stays inside one 512-col PSUM bank.
ST_CHUNKS = {
    0: [(0, 512), (512, 512)],
    1: [(128, 512), (640, 384)],
    2: [(256, 512), (768, 256)],
    3: [(384, 512), (896, 128)],
    4: [(512, 512)],
    5: [(640, 384)],
    6: [(768, 256)],
    7: [(896, 128)],
}
ST_EXT = {t: chunks[-1][0] + chunks[-1][1] - 128 * t for t, chunks in ST_CHUNKS.items()}

DEBUG_DUMP = False


def _emit(nc, tc):
    x_d = nc.dram_tensor("x", [128, 4096], FP16, kind="ExternalInput").ap()
    wqk_d = nc.dram_tensor("wqk", [128, 4096], FP16, kind="ExternalInput").ap()
    wv_d = nc.dram_tensor("wv", [128, 2048], FP16, kind="ExternalInput").ap()
    wp_d = nc.dram_tensor("wp", [128, 2048], FP16, kind="ExternalInput").ap()
    corr_d = nc.dram_tensor("corr", [2, 1664], FP16, kind="ExternalInput").ap()
    bp_d = nc.dram_tensor("bp", [128, 4], F32, kind="ExternalInput").ap()
    y_d = nc.dram_tensor("y", [128, 4096], FP16, kind="ExternalOutput").ap()
    dbg_d = (nc.dram_tensor("dbg", [128, 9224], F32, kind="ExternalOutput").ap()
             if DEBUG_DUMP else None)

    fexp = mybir.ActivationFunctionType.Exp
    fsqrt = mybir.ActivationFunctionType.Sqrt
    fcopy = mybir.ActivationFunctionType.Copy

    with (
        tc.tile_pool(name="pers", bufs=1) as pers,
        tc.tile_pool(name="pT", bufs=2) as ppool,
        tc.tile_pool(name="rsb", bufs=2) as rsb,
    ):
        # ---- persistent SBUF ----
        x_sb = pers.tile([128, 4096], FP16, tag="x")
        sq_sb = pers.tile([128, 4096], FP16, tag="sq")
        wqk_sb = pers.tile([128, 4096], FP16, tag="wqk")
        wv_sb = pers.tile([128, 2048], FP16, tag="wv")
        wp_sb = pers.tile([128, 2048], FP16, tag="wp")
        corr_sb = pers.tile([2, 1664], FP16, tag="corr")
        bp_sb = pers.tile([128, 4], F32, tag="bp")
        q_t = [pers.tile([128, L], FP16, tag=f"q{m}", name=f"q{m}") for m in range(4)]
        k_t = [pers.tile([128, L], FP16, tag=f"k{m}", name=f"k{m}") for m in range(4)]
        # vT_t[m8]: [j, 128h:128h+64] = v^T head h; cols 128h+64:128h+128
        # stay 1.0 so one [128,128] stationary computes AV (rows 0-63) and
        # the softmax denominator (rows 64-127) in a single matmul.
        vT_t = [pers.tile([128, 2 * DM], FP16, tag=f"vT{m}", name=f"vT{m}") for m in range(8)]
        o_t = [pers.tile([128, L], FP16, tag=f"o{m}", name=f"o{m}") for m in range(4)]
        bs_sb = pers.tile([128, L], F32, tag="bs")
        y_sb = pers.tile([128, 4096], FP16, tag="y")
        negmu_row = pers.tile([1, L], FP16, tag="negmu")
        sigma_row = pers.tile([1, L], FP16, tag="sigma")
        var_row = pers.tile([1, L], F32, tag="var")
        mu2_row = pers.tile([1, L], F32, tag="mu2")
        musig = pers.tile([2, L], FP16, tag="musig")
        s16 = pers.tile([128, 8], F32, tag="s16")
        tri2 = pers.tile([128, 256], FP16, tag="tri2")
        ones_row = pers.tile([1, 128], FP16, tag="onesrow")
        ones_col = pers.tile([128, 1], FP16, tag="onescol")
        wsrc = pers.tile([128, 128], FP16, tag="wsrc")

        # ---- input DMAs, one big transfer each, spread across queues ----
        nc.sync.dma_start(x_sb[:, ds(0, 2048)], x_d[:, ds(0, 2048)])
        nc.scalar.dma_start(x_sb[:, ds(2048, 2048)], x_d[:, ds(2048, 2048)])
        nc.gpsimd.memset(wsrc[:], 1.0)
        nc.gpsimd.dma_start(wqk_sb[:], wqk_d[:])
        nc.sync.dma_start(corr_sb[:], corr_d[:])
        nc.sync.dma_start(bp_sb[:], bp_d[:])
        nc.scalar.dma_start(wv_sb[:], wv_d[:])
        nc.sync.dma_start(wp_sb[:], wp_d[:])

        # ---- constants (GpSimd only; it is idle otherwise) ----
        nc.gpsimd.memset(ones_row[:], 1.0)
        nc.gpsimd.memset(ones_col[:], 1.0)
        # tri[p, f] = 1.0 if f >= p else 0.0 (keep i_rel >= j_rel), twice
        # side by side so both heads mask with one 3-dim DVE op.
        nc.gpsimd.memset(tri2[:], 1.0)
        for hh in range(2):
            nc.gpsimd.affine_select(
                out=tri2[:, ds(128 * hh, 128)], in_=tri2[:, ds(128 * hh, 128)],
                compare_op=mybir.AluOpType.is_ge,
                fill=0.0, base=0, pattern=[[1, 128]], channel_multiplier=-1,
            )
        for m8 in range(8):
            nc.gpsimd.memset(vT_t[m8][:], 1.0)

        tri3 = tri2[:].rearrange("p (a b) -> p a b", a=2)

        with (
            tc.tile_pool(name="psW", bufs=1, space="PSUM") as psW,
            tc.tile_pool(name="psM", bufs=1, space="PSUM") as psM,
        ):
            # PE warmup: dep-free matmuls ramp the PE clock to 2.4 GHz
            # while the input DMAs land.
            wu = psW.tile([128, 128], F32, tag="wu")
            for _ in range(24):
                nc.tensor.matmul(wu[:], wsrc[:], wsrc[:], start=True, stop=True)

            # ---- stats: ones-matmul column sums of x and x^2 ----
            for c in range(4):
                nc.vector.tensor_mul(sq_sb[:, ts(c, 1024)],
                                     x_sb[:, ts(c, 1024)], x_sb[:, ts(c, 1024)])
            sum_ps = [psM.tile([1, 512], F32, tag=f"stat{i}", name=f"sum{i}") for i in range(2)]
            sq_ps = [psM.tile([1, 512], F32, tag=f"stat2{i}", name=f"sq{i}") for i in range(2)]
            for chh in range(2):
                for c in range(4):
                    nc.tensor.matmul(
                        sum_ps[chh][:], ones_col[:],
                        x_sb[:, ds(c * 1024 + chh * 512, 512)],
                        start=(c == 0), stop=(c == 3),
                    )
            for chh in range(2):
                for c in range(4):
                    nc.tensor.matmul(
                        sq_ps[chh][:], ones_col[:],
                        sq_sb[:, ds(c * 1024 + chh * 512, 512)],
                        start=(c == 0), stop=(c == 3),
                    )

            # row chain: negmu, var, sigma, s  (all [1, 1024])
            for chh in range(2):
                nc.scalar.activation(negmu_row[:, ts(chh, 512)], sum_ps[chh][:],
                                     fcopy, scale=-1.0 / C)
            nc.vector.tensor_mul(mu2_row[:], negmu_row[:], negmu_row[:])
            for chh in range(2):
                nc.vector.scalar_tensor_tensor(
                    var_row[:, ts(chh, 512)], sq_ps[chh][:], 1.0 / C,
                    mu2_row[:, ts(chh, 512)],
                    mybir.AluOpType.mult, mybir.AluOpType.subtract,
                )
            nc.scalar.activation(sigma_row[:], var_row[:], fsqrt)

            # stack (negmu; sigma) rows for the K=2 correction matmuls
            nc.sync.dma_start(musig[ds(0, 1), :], negmu_row[:])
            nc.scalar.dma_start(musig[ds(1, 1), :], sigma_row[:])

            # broadcast sigma down partitions, then fast full-width recip:
            # bs[p, l] = 1/sigma_l ; s16[p, t] = 1/sigma_(128t+p)
            pbs = psM.tile([128, L], F32, tag="bsb")
            for chh in range(2):
                nc.tensor.matmul(pbs[:, ts(chh, 512)], ones_row[:],
                                 sigma_row[:, ts(chh, 512)],
                                 start=True, stop=True, tile_position=(0, 0))
            for chh in range(2):
                nc.vector.reciprocal_approx_fast(bs_sb[:, ts(chh, 512)],
                                                 pbs[:, ts(chh, 512)])
            ps16 = psM.tile([128, 8], F32, tag="s16p")
            for t in range(8):
                nc.tensor.matmul(ps16[:, ds(t, 1)], sigma_row[:, ds(128 * t, 128)],
                                 ones_row[:, ds(0, 1)],
                                 start=True, stop=True, tile_position=(0, 0))
            nc.vector.reciprocal_approx_fast(s16[:], ps16[:])

        with (
            tc.tile_pool(name="psQ", bufs=2, space="PSUM") as psQ,
            tc.tile_pool(name="psS", bufs=2, space="PSUM") as psS,
            tc.tile_pool(name="psAV", bufs=2, space="PSUM") as psAV,
        ):
            # =========== qkv + attention, interleaved ===========
            def qk_chunk(which, m, chh):
                """q or k chunk [128, 512] -> scaled into q_t/k_t."""
                off = 0 if which == "q" else DM
                dst = q_t[m] if which == "q" else k_t[m]
                ps = psQ.tile([128, 512], F32, tag="qkv", name=f"qkv_{which}{m}_{chh}")
                for c in range(4):
                    nc.tensor.matmul(
                        ps[:],
                        wqk_sb[:, ds(c * 1024 + off + m * 128, 128)],
                        x_sb[:, ds(c * 1024 + chh * 512, 512)],
                        start=(c == 0), stop=False,
                    )
                if which == "q":
                    nc.tensor.matmul(
                        ps[:],
                        corr_sb[:, ds(m * 128, 128)],
                        musig[:, ts(chh, 512)],
                        start=False, stop=True, tile_position=(0, 0),
                    )
                else:
                    nc.tensor.matmul(
                        ps[:],
                        corr_sb[ds(0, 1), ds(512 + m * 128, 128)],
                        musig[ds(0, 1), ts(chh, 512)],
                        start=False, stop=True, tile_position=(0, 0),
                    )
                nc.vector.tensor_mul(dst[:, ts(chh, 512)], ps[:], bs_sb[:, ts(chh, 512)])

            def v_tile(m8):
                """v^T l-tile [128(l), 512(o)] -> scaled into vT_t[m8]."""
                ps = psQ.tile([128, 512], F32, tag="qkv", name=f"v_{m8}")
                for c in range(4):
                    nc.tensor.matmul(
                        ps[:],
                        x_sb[:, ds(c * 1024 + m8 * 128, 128)],
                        wv_sb[:, ds(c * 512, 512)],
                        start=(c == 0), stop=False,
                    )
                nc.tensor.matmul(
                    ps[:],
                    musig[ds(0, 1), ds(128 * m8, 128)],
                    corr_sb[ds(0, 1), ds(1024, 512)],
                    start=False, stop=True, tile_position=(0, 0),
                )
                nc.vector.tensor_scalar_mul(
                    vT_t[m8][:].rearrange("p (h x) -> p h x", x=128)[:, :, ds(0, 64)],
                    ps[:].rearrange("p (h x) -> p h x", x=64),
                    s16[:, ds(m8, 1)],
                )

            pT_all = {}

            def scores(p):
                """scores^T + exp + mask for head pair p; fills pT_all[p]."""
                tiles = {}
                for t in range(8):
                    pT = ppool.tile([128, 2, L], FP16, tag=f"pT{t}", name=f"pT{p}_{t}")
                    for ci, (ist, ncols) in enumerate(ST_CHUNKS[t]):
                        ps = psS.tile([128, 2, 512], F32, tag="sc", name=f"sc{p}_{t}_{ci}")
                        for hh in range(2):
                            pb = 64 * hh
                            nc.tensor.matmul(
                                ps[:, hh, ds(0, ncols)],
                                k_t[p][ds(pb, 64), ts(t, 128)],
                                q_t[p][ds(pb, 64), ds(ist, ncols)],
                                start=True, stop=True,
                                tile_position=(pb, 0),
                            )
                        nc.scalar.activation(
                            pT[:, :, ds(ist - 128 * t, ncols)],
                            ps[:, :, ds(0, ncols)],
                            fexp, scale=0.125,
                        )
                        if ci == 0:
                            # causal mask on the diagonal 128-col block
                            nc.vector.tensor_mul(
                                pT[:, :, ds(0, 128)], pT[:, :, ds(0, 128)], tri3
                            )
                    tiles[t] = pT
                pT_all[p] = tiles

            def av(p, cch):
                """AV + denominator + normalize -> o_t[p] columns cch."""
                tiles = pT_all[p]
                tlist = list(range(4)) if cch == 0 else list(range(8))
                avs = []
                for hh in range(2):
                    h = 2 * p + hh
                    a = psAV.tile([128, 512], F32, tag="av", name=f"av{p}_{cch}_{hh}")
                    avs.append(a)
                    for ti, t in enumerate(tlist):
                        lo = max(cch * 512, 128 * t)
                        n = (cch + 1) * 512 - lo
                        nc.tensor.matmul(
                            a[:, ds(lo - cch * 512, n)],
                            vT_t[t][:, ds(128 * h, 128)],
                            tiles[t][:, hh, ds(lo - 128 * t, n)],
                            start=(ti == 0), stop=(ti == len(tlist) - 1),
                        )
                for hh in range(2):
                    rec = rsb.tile([128, 512], F32, tag="rec", name=f"rec{p}_{cch}_{hh}")
                    nc.vector.reciprocal_approx_fast(rec[:], avs[hh][:])
                    nc.vector.tensor_mul(
                        o_t[p][ds(64 * hh, 64), ts(cch, 512)],
                        avs[hh][ds(0, 64), :], rec[ds(64, 64), :],
                    )

            # PE order: qk(m0) -> v(0..3) -> sc(p0) -> qk(m1) -> sc(p1) ->
            # v(4..7) -> qk(m2) -> sc(p2) -> AV(p0) -> qk(m3) -> sc(p3) ->
            # AV(p1..p3) -> proj.  exp stream starts as soon as sc(p0) is
            # in PSUM; AV(p) trails exp(p).
            for chh in range(2):
                qk_chunk("q", 0, chh)
                qk_chunk("k", 0, chh)
            for m8 in range(4):
                v_tile(m8)
            scores(0)
            for chh in range(2):
                qk_chunk("q", 1, chh)
                qk_chunk("k", 1, chh)
            scores(1)
            for m8 in range(4, 8):
                v_tile(m8)
            for chh in range(2):
                qk_chunk("q", 2, chh)
                qk_chunk("k", 2, chh)
            scores(2)
            av(0, 0)
            av(0, 1)
            for chh in range(2):
                qk_chunk("q", 3, chh)
                qk_chunk("k", 3, chh)
            scores(3)
            av(1, 0)
            av(1, 1)
            av(2, 0)
            av(2, 1)
            av(3, 0)

            # =========== output projection ===========
            def proj(m, chh):
                ps = psAV.tile([128, 512], F32, tag="av", name=f"proj{m}_{chh}")
                for c2 in range(4):
                    nc.tensor.matmul(
                        ps[:],
                        wp_sb[:, ds(c2 * 512 + m * 128, 128)],
                        o_t[c2][:, ts(chh, 512)],
                        start=(c2 == 0), stop=(c2 == 3),
                    )
                nc.vector.tensor_scalar_add(
                    y_sb[:, ds(m * 1024 + chh * 512, 512)], ps[:], bp_sb[:, ds(m, 1)]
                )

            for m in range(4):
                proj(m, 0)
            av(3, 1)
            for m in range(4):
                proj(m, 1)
                nc.sync.dma_start(y_d[:, ds(m * 1024, 1024)], y_sb[:, ds(m * 1024, 1024)])

            if DEBUG_DUMP:
                dbg_sb = pers.tile([128, 9224], F32, tag="dbg")
                nc.gpsimd.memset(dbg_sb[:], 0.0)
                nc.vector.tensor_copy(dbg_sb[:, ds(0, 1024)], q_t[0][:])
                nc.vector.tensor_copy(dbg_sb[:, ds(1024, 1024)], k_t[0][:])
                nc.vector.tensor_copy(dbg_sb[:, ds(2048, 1024)], vT_t[0][:])
                nc.vector.tensor_copy(dbg_sb[:, ds(3072, 1024)], o_t[0][:])
                nc.vector.tensor_copy(dbg_sb[:, ds(4096, 1024)], bs_sb[:])
                nc.vector.tensor_copy(dbg_sb[:, ds(6144, 8)], s16[:])
                nc.vector.tensor_copy(dbg_sb[ds(0, 1), ds(5120, 1024)], negmu_row[:])
                nc.vector.tensor_copy(dbg_sb[ds(0, 1), ds(6152, 1024)], sigma_row[:])
                nc.vector.tensor_copy(dbg_sb[ds(0, 1), ds(7176, 1024)], s_row[:])
                nc.vector.tensor_copy(dbg_sb[ds(0, 1), ds(8200, 1024)], var_row[:])
                nc.sync.dma_start(dbg_d[:], dbg_sb[:])


_NC_CACHE = None


def build_nc():
    global _NC_CACHE
    if _NC_CACHE is None:
        nc = bacc.Bacc("TRN2", target_bir_lowering=False, debug=False)
        with tile.TileContext(nc) as tc:
            _emit(nc, tc)
        nc.compile()
        _NC_CACHE = nc
    return _NC_CACHE


def host_inputs(x, ln_g, ln_b, qkv_w, qkv_b, proj_w, proj_b, pos_h, pos_w):
    """Fold LN affine + v-bias; build per-core input maps."""
    x = np.asarray(x, np.float32)
    ln_g = np.asarray(ln_g, np.float32)
    ln_b = np.asarray(ln_b, np.float32)
    qkv_w = np.asarray(qkv_w, np.float32)
    qkv_b = np.asarray(qkv_b, np.float32)
    proj_w = np.asarray(proj_w, np.float32)
    proj_b = np.asarray(proj_b, np.float32)

    w_eff = qkv_w * ln_g[None, :]                    # [1536, 512]
    b_eff = qkv_b + qkv_w @ ln_b                     # [1536]
    bq, bv = b_eff[:DM], b_eff[2 * DM:]
    bproj = proj_b + proj_w @ bv                     # [512]
    wsum = w_eff.sum(axis=1)                         # [1536]

    def tile128(a, ncols):  # [R, ncols] with R=128*k -> [128, k*ncols]
        k = a.shape[0] // 128
        return np.ascontiguousarray(
            a.reshape(k, 128, ncols).transpose(1, 0, 2).reshape(128, k * ncols)
        )

    wqk = tile128(w_eff[:2 * DM].T, 2 * DM).astype(np.float16)   # [128, 4096]
    wv = tile128(w_eff[2 * DM:].T, DM).astype(np.float16)        # [128, 2048]
    wp = tile128(proj_w.T, DM).astype(np.float16)                # [128, 2048]

    corr = np.zeros((2, 1664), np.float32)
    corr[0, 0:512] = wsum[:DM]            # wsumq
    corr[0, 512:1024] = wsum[DM:2 * DM]   # wsumk
    corr[0, 1024:1536] = wsum[2 * DM:]    # wsumv
    corr[1, 0:512] = bq
    corr = corr.astype(np.float16)

    bp = np.ascontiguousarray(bproj.reshape(4, 128).T)           # [128, 4] f32

    common = {
        "wqk": wqk, "wv": wv, "wp": wp, "corr": corr, "bp": bp,
    }
    in_maps = []
    for b in range(B):
        xb = x[b].reshape(C, L)
        m = dict(common)
        m["x"] = tile128(xb, L).astype(np.float16)               # [128, 4096]
        in_maps.append(m)
    return in_maps


def kernel(x, ln_g, ln_b, qkv_w, qkv_b, proj_w, proj_b, pos_h, pos_w, **kw):
    nc = build_nc()
    in_maps = host_inputs(x, ln_g, ln_b, qkv_w, qkv_b, proj_w, proj_b, pos_h, pos_w)
    res = run_bass_kernel_spmd(nc, in_maps, core_ids=list(range(NCORES)))
    out = np.empty((B, C, H, W), np.float32)
    for b in range(B):
        yb = res.results[b]["y"].astype(np.float32)              # [128, 4096]
        out[b] = yb.reshape(128, 4, L).transpose(1, 0, 2).reshape(C, H, W)
    return out


if __name__ == "__main__":
    nc = build_nc()
    print("built + compiled ok")
